# revision 17
# baseline (speedup 1.0000x reference)
"""Trainium2 Bass kernel: top-2 MoE (8 experts, E=1024, H=1536, T=16384).

Sharding: data-parallel over the batch axis -- each of the 8 NeuronCores
processes one batch row (2048 tokens) end to end:
  1. fp32 router on device (logits matmul, softmax, top-2 via threshold mask)
  2. on-device stream compaction (gpsimd sparse_gather) -> per-expert token
     lists; the dispatch metadata never round-trips through DRAM:
     masked token-id planes are moved to the 16-partition sparse_gather
     layout with a TensorE transpose, and the compacted list is replicated
     to all 128 partitions with a tiled-identity matmul
  3. dma_gather(transpose=True) pulls each expert's token rows from HBM in
     bf16, already transposed to feature-major for the matmuls
  4. per-expert FFN at a static capacity of 640 tokens (actual max per-expert
     count for the routed input is checked on host):
     H^T = gelu(W1^T X^T + b1); then token-major Y via stationary H^T tiles
  5. gating (softmax prob of the selected expert) applied as a per-partition
     ACT scale while evacuating PSUM
  6. dma_scatter_add accumulates gated bf16 rows into the bf16 output (the
     ExternalOutput buffer is pre-zeroed by the runtime)

Token rows in DRAM (xbf / gating table / out) are staged in "r-major" order
r = (t % 128) * 16 + t // 128 so the on-device gating-table store is one
contiguous-descriptor DMA; the host un-permutes the output rows.

Host work is limited to sharding/staging (slice, transpose, bf16 cast of the
staged copies) and a capacity-safety check; all arithmetic producing the
output runs on the NeuronCores.
"""

import numpy as np
import ml_dtypes

import concourse.bacc as bacc
import concourse.mybir as mybir
import concourse.tile as tile
from concourse.alu_op_type import AluOpType
from concourse.bass_utils import run_bass_kernel_spmd
from concourse.tile_rust import add_dep_helper

F32 = mybir.dt.float32
BF16 = mybir.dt.bfloat16
I16 = mybir.dt.int16
U32 = mybir.dt.uint32
AF = mybir.ActivationFunctionType

B, N, E, H, NE = 8, 2048, 1024, 1536, 8
KT = E // 128          # 8 k-tiles of x features
HT = H // 128          # 12 tiles of hidden
C = 640                # per-expert token capacity (multiple of 128)
CT = C // 128          # 5 token tiles per expert
CW = C // 16           # wrapped idx columns
NP = N + 128           # gather/scatter tables padded with a zero dummy row
SGF = 128 + CW         # sparse_gather free dim: 2048 real slots + C dummies

_CACHE = {}


def _build_nc():
    nc = bacc.Bacc("TRN2", target_bir_lowering=False)

    xT = nc.dram_tensor("xT", [E, N], F32, kind="ExternalInput")
    xbf = nc.dram_tensor("xbf", [NP, E], BF16, kind="ExternalInput")
    wr = nc.dram_tensor("wr", [E, NE], F32, kind="ExternalInput")
    w1 = nc.dram_tensor("w1", [NE, E, H], BF16, kind="ExternalInput")
    w2 = nc.dram_tensor("w2", [NE, H, E], BF16, kind="ExternalInput")
    tok1 = nc.dram_tensor("tok1", [128, 16, 1], F32, kind="ExternalInput")
    eye8 = nc.dram_tensor("eye8", [8, 8], F32, kind="ExternalInput")
    eye128 = nc.dram_tensor("eye128", [128, 128], F32, kind="ExternalInput")
    brv = nc.dram_tensor("brv", [8, 1], F32, kind="ExternalInput")
    b1v = nc.dram_tensor("b1v", [128, NE, HT], F32, kind="ExternalInput")
    out = nc.dram_tensor("out", [NP, E], BF16, kind="ExternalOutput")

    gat_d = nc.dram_tensor("gat_d", [NP, 64], F32)
    lists_d = nc.dram_tensor("lists_d", [NE, 16, CW], I16)

    with tile.TileContext(nc) as tc:
        with (
            tc.tile_pool(name="consts", bufs=1) as cpool,
            tc.tile_pool(name="lists", bufs=2) as lpool,
            tc.tile_pool(name="xg", bufs=2) as xg_pool,
            tc.tile_pool(name="gt", bufs=2) as gt_pool,
            tc.tile_pool(name="w1p", bufs=2) as w1_pool,
            tc.tile_pool(name="w2p", bufs=2) as w2_pool,
            tc.tile_pool(name="hT", bufs=1) as h_pool,
            tc.tile_pool(name="y", bufs=2) as y_pool,
            tc.tile_pool(name="psH", bufs=2, space="PSUM") as psH_pool,
            tc.tile_pool(name="psY", bufs=2, space="PSUM") as psY_pool,
        ):
            # ---- constants ----
            wr_sb = cpool.tile([128, KT, NE], F32)
            nc.sync.dma_start(wr_sb[:], wr.rearrange("(k p) c -> p k c", p=128))
            eye_sb = cpool.tile([8, 8], F32)
            nc.sync.dma_start(eye_sb[:], eye8[:])
            eye128_sb = cpool.tile([128, 128], F32)
            nc.sync.dma_start(eye128_sb[:], eye128[:])
            tok1_sb = cpool.tile([128, 16, 1], F32)
            nc.sync.dma_start(tok1_sb[:], tok1[:])
            brv_sb = cpool.tile([8, 1], F32)
            nc.sync.dma_start(brv_sb[:], brv[:])
            b1_sb = cpool.tile([128, NE, HT], F32)
            nc.sync.dma_start(b1_sb[:], b1v[:])

            rpool_cm = tc.tile_pool(name="router_sb", bufs=1)
            xt_pool_cm = tc.tile_pool(name="router_x", bufs=2)
            idx_sbs = []
            with rpool_cm as rpool, xt_pool_cm as xt_pool:
                # ---- router: logits^T [8, N] = Wr^T @ X^T (+ br), fp32 ----
                ltr = rpool.tile([8, N], F32)
                last_router_mm = None
                with tc.tile_pool(name="router_ps", bufs=1, space="PSUM") as psL_pool:
                    psL = [psL_pool.tile([8, 512], F32, tag=f"psL{i}",
                                         name=f"psL{i}")
                           for i in range(4)]
                    for k in range(KT):
                        xt_sb = xt_pool.tile([128, N], F32)
                        # split each k-tile into column halves across the two
                        # HWDGE rings (SP / ACT): first matmul starts sooner
                        # and the router stays DMA-paced at full HBM rate
                        for h2 in range(2):
                            eng = nc.sync if (2 * k + h2) % 2 == 0 else nc.scalar
                            eng.dma_start(
                                xt_sb[:, 1024 * h2:1024 * (h2 + 1)],
                                xT[128 * k:128 * (k + 1),
                                   1024 * h2:1024 * (h2 + 1)])
                        for c4 in range(4):
                            last_router_mm = nc.tensor.matmul(
                                psL[c4][:],
                                lhsT=wr_sb[:, k, :],
                                rhs=xt_sb[:, 512 * c4:512 * (c4 + 1)],
                                start=(k == 0),
                                stop=(k == KT - 1),
                            )
                    for c4 in range(4):
                        nc.scalar.activation(
                            ltr[:, 512 * c4:512 * (c4 + 1)], psL[c4][:],
                            AF.Identity, bias=brv_sb[:],
                        )

                # ---- transpose logits to token-major [128, 16*8] ----
                ltm = rpool.tile([128, 16, NE], F32)
                with tc.tile_pool(name="psT", bufs=1, space="PSUM") as psT_pool:
                    psT = psT_pool.tile([128, 128], F32)
                    for bi in range(16):
                        nc.tensor.transpose(
                            out=psT[:, 8 * bi:8 * (bi + 1)],
                            in_=ltr[:, 128 * bi:128 * (bi + 1)],
                            identity=eye_sb[:],
                        )
                    nc.vector.tensor_copy(ltm[:], psT[:])

                # ---- top-2 selection on raw fp32 logits (keeps the exp LUT
                # out of the selection path; softmax is monotone so top-2 by
                # logits == top-2 by probs) ----
                rmax = rpool.tile([128, 16, 1], F32)
                nc.vector.tensor_reduce(rmax[:], ltm[:], axis=mybir.AxisListType.X,
                                        op=AluOpType.max)
                ismax = rpool.tile([128, 16, NE], F32)
                nc.vector.tensor_tensor(ismax[:], ltm[:],
                                        rmax[:].to_broadcast([128, 16, NE]),
                                        op=AluOpType.is_ge)
                masked2 = rpool.tile([128, 16, NE], F32)
                nc.vector.scalar_tensor_tensor(masked2[:], in0=ismax[:],
                                               scalar=-1.0e5, in1=ltm[:],
                                               op0=AluOpType.mult,
                                               op1=AluOpType.add)
                thr = rpool.tile([128, 16, 1], F32)
                nc.vector.tensor_reduce(thr[:], masked2[:],
                                        axis=mybir.AxisListType.X,
                                        op=AluOpType.max)
                mask = rpool.tile([128, 16, NE], F32)
                nc.vector.tensor_tensor(mask[:], ltm[:],
                                        thr[:].to_broadcast([128, 16, NE]),
                                        op=AluOpType.is_ge)

                # masked token-id planes, one per expert (r-major ids)
                midx = rpool.tile([128, 16, NE], F32)
                nc.vector.tensor_tensor(midx[:], mask[:],
                                        tok1_sb[:].to_broadcast([128, 16, NE]),
                                        op=AluOpType.mult)
                nc.vector.tensor_scalar_add(midx[:], midx[:], -1.0)

                # ---- per-expert compaction ----
                # HW sparse_gather writes garbage beyond num_found, so C dummy
                # slots (value N = dummy token) are appended to the *input*:
                # the compacted output then always starts with the real tokens
                # followed by dummies, making the first C slots deterministic.
                # The masked token-id planes reach the 16-partition
                # sparse_gather layout via a TensorE transpose (no DRAM hop);
                # the compacted list is replicated to all 128 partitions
                # through a small lists_d store + 8 loads.
                sg_insts = []
                with tc.tile_pool(name="psD", bufs=3, space="PSUM") as psD_pool:
                    for e in range(NE):
                        # midx[:, :, e] [128,16] -> [16,128] on partitions 0-15
                        psd = psD_pool.tile([16, 128], F32, tag="psd")
                        nc.tensor.transpose(out=psd[:], in_=midx[:, :, e],
                                            identity=eye128_sb[:])
                        sg_in = lpool.tile([16, SGF], F32, tag="sg_in",
                                           bufs=NE)
                        nc.vector.memset(sg_in[:], float(N))
                        nc.vector.tensor_copy(sg_in[:, 0:128], psd[:])
                        slist = lpool.tile([16, SGF], F32, tag="slist")
                        nfound = lpool.tile([1, 1], U32, tag="nfound")
                        sg_i = nc.gpsimd.sparse_gather(slist[:], sg_in[:],
                                                       num_found=nfound[:])
                        sg_insts.append(sg_i)
                        # list replication via DRAM, on the ACT ring (the SP
                        # ring is stuffed with the deferred weight loads)
                        ilist = lpool.tile([16, CW], I16, tag="ilist")
                        nc.vector.tensor_copy(ilist[:], slist[:, 0:CW])
                        nc.scalar.dma_start(lists_d[e], ilist[:])
                        idx_sb = lpool.tile([128, CW], I16, tag=f"idx{e}",
                                            bufs=1)
                        for g in range(8):
                            nc.scalar.dma_start(idx_sb[16 * g:16 * (g + 1), :],
                                                lists_d[e])
                        idx_sbs.append(idx_sb)

                # ---- softmax probs (gating values only) ----
                cmb = rpool.tile([128, 16, NE], F32)
                nc.vector.tensor_sub(cmb[:], ltm[:],
                                     rmax[:].to_broadcast([128, 16, NE]))
                nc.scalar.activation(cmb[:], cmb[:], AF.Exp)
                esum = rpool.tile([128, 16, 1], F32)
                nc.vector.tensor_reduce(esum[:], cmb[:], axis=mybir.AxisListType.X,
                                        op=AluOpType.add)
                rs = rpool.tile([128, 16, 1], F32)
                nc.vector.reciprocal(rs[:], esum[:])
                nc.vector.tensor_tensor(cmb[:], cmb[:],
                                        rs[:].to_broadcast([128, 16, NE]),
                                        op=AluOpType.mult)

                # gating table: row r = p*16 + bi -> 4KB contiguous per
                # partition on both sides (token rows zero-padded to 64 floats
                # so dma_gather's 256B-aligned rows stay fully initialized)
                cmb64 = rpool.tile([128, 16, 64], F32)
                nc.vector.memset(cmb64[:], 0.0)
                nc.vector.tensor_copy(cmb64[:, :, 0:NE], cmb[:])
                nc.scalar.dma_start(
                    gat_d[0:N].rearrange("(p bi) c -> p bi c", p=128), cmb64[:])
                zrow = rpool.tile([128, 64], F32)
                nc.vector.memset(zrow[:], 0.0)
                nc.scalar.dma_start(gat_d[N:NP, :], zrow[:])

            # ---- per-expert FFN (mlp library: dma_gather / dma_scatter_add) ----
            xg_insts, gt_insts, sc_insts = [], [], []
            for e in range(NE):
                xg = xg_pool.tile([128, KT, C], BF16)
                xg_i = nc.gpsimd.dma_gather(
                    out_ap=xg[:], in_ap=xbf[:], idxs_ap=idx_sbs[e][:],
                    num_idxs=C, num_idxs_reg=C, elem_size=E, transpose=True)
                xg_insts.append(xg_i)
                gt = gt_pool.tile([128, CT, 64], F32)
                gt_i = nc.gpsimd.dma_gather(
                    out_ap=gt[:], in_ap=gat_d[:], idxs_ap=idx_sbs[e][:],
                    num_idxs=C, num_idxs_reg=C, elem_size=64, transpose=False)
                gt_insts.append(gt_i)

                w1_sb = w1_pool.tile([128, KT, H], BF16)
                w1_ld = nc.sync.dma_start(
                    w1_sb[:], w1[e].rearrange("(k p) h -> p k h", p=128))
                w2_sb = w2_pool.tile([128, HT, E], BF16)
                w2_ld = nc.sync.dma_start(
                    w2_sb[:], w2[e].rearrange("(k p) f -> p k f", p=128))
                if e == 0:
                    # keep the startup HBM bandwidth exclusively for the
                    # router input: weight streaming starts only once the
                    # last router matmul has consumed xT
                    add_dep_helper(w1_ld.ins, last_router_mm.ins, sync=True,
                                   reason="defer weights past router xT")
                    add_dep_helper(w2_ld.ins, last_router_mm.ins, sync=True,
                                   reason="defer weights past router xT")

                hT = h_pool.tile([128, HT, C], BF16)
                for h in range(HT):
                    for c0, cw in ((0, 512), (512, 128)):
                        ps = psH_pool.tile([128, cw], F32, tag="psH")
                        for k in range(KT):
                            nc.tensor.matmul(
                                ps[:], lhsT=w1_sb[:, k, 128 * h:128 * (h + 1)],
                                rhs=xg[:, k, c0:c0 + cw],
                                start=(k == 0), stop=(k == KT - 1))
                        nc.scalar.activation(hT[:, h, c0:c0 + cw], ps[:],
                                             AF.Gelu, bias=b1_sb[:, e, h:h + 1])

                y_sb = y_pool.tile([128, CT, E], BF16)
                for tt in range(CT):
                    for n2 in range(2):
                        ps = psY_pool.tile([128, 512], F32, tag="psY")
                        for k2 in range(HT):
                            nc.tensor.matmul(
                                ps[:], lhsT=hT[:, k2, 128 * tt:128 * (tt + 1)],
                                rhs=w2_sb[:, k2, 512 * n2:512 * (n2 + 1)],
                                start=(k2 == 0), stop=(k2 == HT - 1))
                        nc.scalar.activation(
                            y_sb[:, tt, 512 * n2:512 * (n2 + 1)], ps[:],
                            AF.Copy, scale=gt[:, tt, e:e + 1])

                if e < NE - 1:
                    sc_i = nc.gpsimd.dma_scatter_add(
                        out_ap=out[:], in_ap=y_sb[:], idxs_ap=idx_sbs[e][:],
                        num_idxs=C, num_idxs_reg=C, elem_size=E)
                    sc_insts.append(sc_i)
                else:
                    # split the final scatter so the kernel-tail drain only
                    # waits on the last 2 token tiles
                    sc_a = nc.gpsimd.dma_scatter_add(
                        out_ap=out[:], in_ap=y_sb[:, 0:3, :],
                        idxs_ap=idx_sbs[e][:, 0:24],
                        num_idxs=384, num_idxs_reg=384, elem_size=E)
                    sc_b = nc.gpsimd.dma_scatter_add(
                        out_ap=out[:], in_ap=y_sb[:, 3:5, :],
                        idxs_ap=idx_sbs[e][:, 24:CW],
                        num_idxs=256, num_idxs_reg=256, elem_size=E)
                    sc_insts.append(sc_a)
                    sc_insts.append(sc_b)

            # ---- pin the gpsimd custom-op order ----
            # The sparse_gather ucode and the dma_gather/scatter ucode live in
            # different gpsimd libraries; each alternation costs a ~6us
            # library swap + IRAM refetch. Order ops to (a) get expert 0's
            # gathers started as early as possible, (b) batch the remaining
            # sparse_gathers in one library session, (c) keep later gathers
            # ahead of scatters so FFN inputs are never starved.
            order = [sg_insts[0], xg_insts[0], gt_insts[0]]
            order += sg_insts[1:]
            order += [xg_insts[1], gt_insts[1]]
            for e in range(2, NE):
                order += [sc_insts[e - 2], xg_insts[e], gt_insts[e]]
            order += sc_insts[NE - 2:]
            for a, b in zip(order[1:], order):
                add_dep_helper(a.ins, b.ins, sync=False,
                               reason="gpsimd op order")

    return nc


def get_nc():
    if "nc" not in _CACHE:
        nc = _build_nc()
        nc.finalize()  # Bacc.compile(): reg alloc, library-load insertion, ...
        _CACHE["nc"] = nc
    return _CACHE["nc"]


def make_in_maps(inputs):
    x = np.asarray(inputs["x"], dtype=np.float32)
    Wr = np.asarray(inputs["Wr"], dtype=np.float32)
    br = np.asarray(inputs["br"], dtype=np.float32)
    W1 = np.asarray(inputs["W1"], dtype=np.float32)
    b1 = np.asarray(inputs["b1"], dtype=np.float32)
    W2 = np.asarray(inputs["W2"], dtype=np.float32)
    b2 = np.asarray(inputs["b2"], dtype=np.float32)
    assert x.shape == (B, N, E) and W1.shape == (NE, E, H) and W2.shape == (NE, H, E)
    if b2.any():
        raise NotImplementedError("nonzero b2 path not emitted in this kernel")

    # capacity guard: the kernel is compiled for a static per-expert capacity
    # of C tokens per core; verify the actual routing fits.
    logits = x.reshape(B * N, E) @ Wr + br
    part = np.partition(logits, NE - 2, axis=-1)[:, NE - 2:NE - 1]
    sel = logits >= part
    counts = sel.reshape(B, N, NE).sum(1)
    if counts.max() > C:
        raise RuntimeError(f"expert capacity exceeded: {counts.max()} > {C}")

    bf = ml_dtypes.bfloat16
    # r-major token ids: token t = bi*128 + p lives in DRAM row r = p*16 + bi
    tok1 = (np.arange(128)[:, None] * 16 + np.arange(16)[None, :] + 1.0)
    tok1 = tok1.astype(np.float32).reshape(128, 16, 1)
    eye8 = np.eye(8, dtype=np.float32)
    eye128 = np.eye(128, dtype=np.float32)
    brv = br.reshape(NE, 1).astype(np.float32)
    # b1v[p, e, h] = b1[e, h*128 + p]
    b1v = np.ascontiguousarray(b1.reshape(NE, HT, 128).transpose(2, 0, 1))
    W1b = W1.astype(bf)
    W2b = W2.astype(bf)

    in_maps = []
    for c in range(B):
        # xr[r] = x[c][t] with r = (t % 128)*16 + t//128
        xr = x[c].reshape(16, 128, E).transpose(1, 0, 2).reshape(N, E)
        in_maps.append({
            "xT": np.ascontiguousarray(x[c].T),
            "xbf": np.concatenate(
                [xr, np.zeros((NP - N, E), np.float32)], axis=0).astype(bf),
            "wr": Wr,
            "w1": W1b,
            "w2": W2b,
            "tok1": tok1,
            "eye8": eye8,
            "eye128": eye128,
            "brv": brv,
            "b1v": b1v,
        })
    return in_maps


def run(inputs, **kw):
    in_maps = make_in_maps(inputs)
    nc = get_nc()
    res = run_bass_kernel_spmd(nc, in_maps, list(range(B)), **kw)
    outs = []
    for c in range(B):
        out_r = np.asarray(res.results[c]["out"][0:N], dtype=np.float32)
        # un-permute: out[t] = out_r[(t % 128)*16 + t//128]
        outs.append(out_r.reshape(128, 16, E).transpose(1, 0, 2).reshape(N, E))
    return np.stack(outs, axis=0), res


def kernel(**inputs):
    out, _ = run(inputs)
    return out
